# revision 1
# baseline (speedup 1.0000x reference)
"""Capsule routing pooling kernel for Trainium2 (8 NeuronCores, data parallel).

Math: the reference's softmax is over a singleton axis, so the routing
coefficients are identically 1.0 and the routing iterations never affect the
output.  The computation reduces to, per (b, c, 2x2 spatial tile):
    s   = sum of the four D=16 vectors in the tile
    sq  = sum_d s_d^2
    out = s * sq / ((1 + sq) * (sqrt(sq) + 1e-8))

Sharding: batch dim (16) split across 8 cores -> 2 batches/core.  Per core the
(2*64)=128 (b,c) pairs map onto the 128 SBUF partitions; each partition owns a
full 64x64x16 image.

Per-core pipeline (groups of G=4 output-row pairs):
  - even input rows DMA'd into SBUF (HWDGE), odd rows DMA'd with
    accum_op=add (SWDGE) -> row-pair sums come out of the DMA engines free
  - column-pair add, square (ACT), reduce over D, squash scale chain, final
    multiply, store.
"""

import numpy as np

import concourse.bass as bass
import concourse.bacc as bacc
import concourse.tile as tile
from concourse import mybir
from concourse.bass_utils import run_bass_kernel_spmd

_B, _C, _H, _W, _D = 16, 64, 64, 64, 16
_NCORES = 8
_F32 = mybir.dt.float32


def _kernel_body(tc, out_ap, in_ap, H, W, D, G):
    nc = tc.nc
    P = 128
    nH, nW = H // 2, W // 2
    NG = nH // G
    N = G * nW  # squash vectors per group per partition

    inv = in_ap.rearrange("p (h2 two) w d -> p h2 two w d", two=2)

    import contextlib

    with contextlib.ExitStack() as ctx:
        big = ctx.enter_context(tc.tile_pool(name="big", bufs=3))
        mid = ctx.enter_context(tc.tile_pool(name="mid", bufs=2))
        outp = ctx.enter_context(tc.tile_pool(name="outp", bufs=3))
        small = ctx.enter_context(tc.tile_pool(name="small", bufs=2))

        for g in range(NG):
            # row-pair sums r[p, gi, x', d] for rows 2*(G*g+gi), 2*(G*g+gi)+1
            r = big.tile([P, G, W, D], _F32, tag="r")
            nc.sync.dma_start(out=r[:], in_=inv[:, G * g : G * (g + 1), 0, :, :])
            nc.gpsimd.dma_start(
                out=r[:],
                in_=inv[:, G * g : G * (g + 1), 1, :, :],
                accum_op=mybir.AluOpType.add,
            )

            # column-pair add -> s
            rv = r[:].rearrange("p g (x two) d -> p g x two d", two=2)
            s = mid.tile([P, G, nW, D], _F32, tag="s")
            nc.vector.tensor_add(s[:], rv[:, :, :, 0, :], rv[:, :, :, 1, :])

            # sq = sum_d s^2
            s2 = mid.tile([P, G, nW, D], _F32, tag="s2")
            nc.scalar.activation(s2[:], s[:], mybir.ActivationFunctionType.Square)
            sq = small.tile([P, N, 1], _F32, tag="sq")
            nc.vector.tensor_reduce(
                sq[:],
                s2[:].rearrange("p g x d -> p (g x) d"),
                axis=mybir.AxisListType.X,
                op=mybir.AluOpType.add,
            )

            # scale = sq / ((1 + sq) * (sqrt(sq) + 1e-8))
            c1 = small.tile([P, N, 1], _F32, tag="c1")
            nc.vector.tensor_scalar_add(c1[:], sq[:], 1.0)
            a = small.tile([P, N, 1], _F32, tag="a")
            nc.scalar.activation(a[:], sq[:], mybir.ActivationFunctionType.Sqrt)
            den = small.tile([P, N, 1], _F32, tag="den")
            nc.vector.scalar_tensor_tensor(
                den[:],
                a[:],
                1e-8,
                c1[:],
                op0=mybir.AluOpType.add,
                op1=mybir.AluOpType.mult,
            )
            rec = small.tile([P, N, 1], _F32, tag="rec")
            nc.vector.reciprocal(rec[:], den[:])
            sc = small.tile([P, N, 1], _F32, tag="sc")
            nc.vector.tensor_mul(sc[:], sq[:], rec[:])

            # out = s * scale (broadcast over D)
            o = outp.tile([P, G, nW, D], _F32, tag="o")
            nc.vector.tensor_mul(
                o[:].rearrange("p g x d -> p (g x) d"),
                s[:].rearrange("p g x d -> p (g x) d"),
                sc[:].to_broadcast((P, N, D)),
            )
            nc.sync.dma_start(out=out_ap[:, G * g : G * (g + 1), :, :], in_=o[:])


def build_nc(H=_H, W=_W, D=_D, G=4):
    """Build and compile the per-core Bass program."""
    nc = bacc.Bacc("TRN2", target_bir_lowering=False, debug=False)
    inp = nc.dram_tensor("inp", [128, H, W, D], _F32, kind="ExternalInput").ap()
    out = nc.dram_tensor(
        "out", [128, H // 2, W // 2, D], _F32, kind="ExternalOutput"
    ).ap()
    with tile.TileContext(nc) as tc:
        _kernel_body(tc, out, inp, H, W, D, G)
    nc.compile()
    return nc


_NC_CACHE = {}


def _get_nc():
    if "nc" not in _NC_CACHE:
        _NC_CACHE["nc"] = build_nc()
    return _NC_CACHE["nc"]


def kernel(inp, kernel_size=2, routing_iteration=3, _trace=False):
    inp = np.asarray(inp, dtype=np.float32)
    assert int(kernel_size) == 2, "kernel compiled for kernel_size=2"
    assert inp.shape == (_B, _C, _H, _W, _D), inp.shape
    # routing_iteration is mathematically irrelevant (softmax over singleton
    # axis -> coefficients identically 1); any value >= 1 gives this output.

    nc = _get_nc()
    bpc = _B // _NCORES  # batches per core
    in_maps = [
        {"inp": np.ascontiguousarray(inp[i * bpc : (i + 1) * bpc]).reshape(128, _H, _W, _D)}
        for i in range(_NCORES)
    ]
    res = run_bass_kernel_spmd(
        nc, in_maps, core_ids=list(range(_NCORES)), trace=_trace
    )
    out = np.empty((_B, _C, _H // 2, _W // 2, _D), dtype=np.float32)
    for i in range(_NCORES):
        out[i * bpc : (i + 1) * bpc] = res.results[i]["out"].reshape(
            bpc, _C, _H // 2, _W // 2, _D
        )
    if _trace:
        return out, res
    return out
